# revision 23
# baseline (speedup 1.0000x reference)
"""Trainium2 Bass kernel for nn_CliffordFourierHead (CGENN-style Clifford net).

Network (per reference): B=1024, IN=256, HID=512, OUT=128, Cl(3,0), 8 blades.
  fcgp1 -> MVSiLU -> channel-wise steerable GP -> MVSiLU -> fcgp2

Strategy (v3):
  - Pure batch data-parallelism over 8 NeuronCores (128 batch rows each).
  - Channels on partitions, batch on free dim; an activation is 8 blade
    planes packed into one [128, 8*128] SBUF tile per channel-tile.
  - Geometric products: ONE wide DVE op pair builds a mega product tile
    Q[i,k] = x_i * xr_k (64 blade-pair planes); the Cayley contraction is
    absorbed into TensorE via strided plane-selection rhs APs accumulating
    into PSUM (negative signs via negated weight copies built on idle
    engines).
  - PSUM groups are single [128, 1024] f32 tiles spanning 2 banks
    (one start per bank) -> 4 groups in flight for deep pipelining.
  - Normalize / MVSiLU run as few wide ops: Abs for 1-blade grades,
    single wide squares from PSUM, fused sigmoid scale/bias (folds the
    scalar-blade bias), and gating multiplies read PSUM directly -- no
    evacuation copies.
  - Weights prefetched one phase-pair ahead; fp16 on-chip compute.

Self-contained: shapes and the Cl(3,0) Cayley table are derived inline.
"""

import contextlib
import math

import numpy as np

NCORES = 8
B, NIN, HID, NOUT = 1024, 256, 512, 128
BC = B // NCORES  # 128 batch rows per core
NB = 8
KT_IN, KT_HID = NIN // 128, HID // 128  # 2, 4
MT_IN, MT_HID, MT_OUT = NIN // 128, HID // 128, NOUT // 128  # 2, 4, 1
GRADE_SLICES = [(0, 1), (1, 4), (4, 7), (7, 8)]
GW = [1, 3, 3, 1]
EPS = 1e-6
ISQ2 = 1.0 / math.sqrt(2.0)


def _build_cayley():
    masks = sorted(range(NB), key=lambda m: (bin(m).count("1"), m))
    pos = {m: i for i, m in enumerate(masks)}
    cay = np.zeros((NB, NB, NB), dtype=np.float32)
    for i, mi in enumerate(masks):
        for k, mk in enumerate(masks):
            a, s = mi >> 1, 0
            while a:
                s += bin(a & mk).count("1")
                a >>= 1
            cay[i, pos[mi ^ mk], k] = -1.0 if (s & 1) else 1.0
    triples = []
    for gi in range(4):
        for gj in range(4):
            for gk in range(4):
                (i0, i1), (j0, j1), (k0, k1) = (
                    GRADE_SLICES[gi], GRADE_SLICES[gj], GRADE_SLICES[gk])
                if np.any(cay[i0:i1, j0:j1, k0:k1] != 0):
                    triples.append((gi, gj, gk))
    return cay, triples


CAY, TRIPLES = _build_cayley()
NPATHS = len(TRIPLES)  # 20

# Per triple t: {j: [(i, k, sign), ...]}
TRIPLE_TERMS = []
for t, (gi, gj, gk) in enumerate(TRIPLES):
    (i0, i1), (k0, k1) = GRADE_SLICES[gi], GRADE_SLICES[gk]
    d = {}
    for i in range(i0, i1):
        for k in range(k0, k1):
            j = int(np.nonzero(CAY[i, :, k])[0][0])
            if GRADE_SLICES[gj][0] <= j < GRADE_SLICES[gj][1]:
                d.setdefault(j, []).append((i, k, float(CAY[i, j, k])))
    TRIPLE_TERMS.append(d)


def _build_term_sets():
    """Per triple: list of matmul term-sets (j0, L, plane0, plane_step, sign).

    A term-set is a run of consecutive output blades j0..j0+L-1, one product
    plane each, with uniform Cayley sign and arithmetic plane offsets
    (plane = i*8+k in the mega product tile) -> a single matmul with a
    strided rhs plane selection into psum columns [j0*BC, (j0+L)*BC).
    """
    all_sets = []
    for t in range(NPATHS):
        terms = []
        for j, lst in TRIPLE_TERMS[t].items():
            for (i, k, s) in lst:
                terms.append((j, i * 8 + k, s))
        sets = []
        for sgn in (1.0, -1.0):
            pool = sorted(x for x in terms if x[2] == sgn)
            while pool:
                j0, o0, _ = pool.pop(0)
                run = [(j0, o0)]
                step = None
                while True:
                    pick = None
                    for c in pool:
                        if c[0] != run[-1][0] + 1:
                            continue
                        st = c[1] - run[-1][1]
                        if step is None or st == step:
                            pick, pstep = c, st
                            break
                    if pick is None:
                        break
                    step = pstep
                    pool.remove(pick)
                    run.append((pick[0], pick[1]))
                sets.append((run[0][0], len(run), run[0][1], step or 0, sgn))
        all_sets.append(sets)
    return all_sets


TERM_SETS = _build_term_sets()
GP_SETS = [(t, s) for t in range(NPATHS) for s in TERM_SETS[t]]
NEG_TRIPLES = sorted({t for t in range(NPATHS)
                      if any(s[4] < 0 for s in TERM_SETS[t])})
NEG_SLOT = {t: n for n, t in enumerate(NEG_TRIPLES)}
NNEG = len(NEG_TRIPLES)
# consecutive runs of NEG_TRIPLES, for on-chip negation ops
NEG_RUNS = []
_i = 0
while _i < NNEG:
    _j = _i
    while _j + 1 < NNEG and NEG_TRIPLES[_j + 1] == NEG_TRIPLES[_j] + 1:
        _j += 1
    NEG_RUNS.append((NEG_TRIPLES[_i], _j - _i + 1))
    _i = _j + 1


# ----------------------------------------------------------------------------
# Host-side prep
# ----------------------------------------------------------------------------
def prep_in_maps(inputs):
    f16, f32 = np.float16, np.float32

    def lin_w(w, scale=1.0):
        m, n, _ = np.asarray(w).shape
        wt = np.transpose(np.asarray(w, f32), (1, 2, 0))  # [n, 4, m]
        wt = wt.reshape(n // 128, 128, 4, m).transpose(0, 2, 1, 3)
        return np.ascontiguousarray(wt * scale).astype(f16)  # [kt, g, 128, m]

    def gp_w(w, scale):
        m, n, _ = np.asarray(w).shape
        wt = np.transpose(np.asarray(w, f32), (2, 1, 0))  # [20, n, m]
        wt = (wt.reshape(NPATHS, n // 128, 128, m) * scale).transpose(
            1, 0, 2, 3)                                    # [kt, t, 128, m]
        wt = np.ascontiguousarray(wt).astype(f16)
        wn = np.ascontiguousarray(-wt[:, NEG_TRIPLES])     # [kt, nneg, 128, m]
        return wt, wn

    def sig(a):
        return 1.0 / (1.0 + np.exp(-np.asarray(a, f32)))

    x = np.asarray(inputs["x"], f32)

    c = {}
    c["lr1w"] = lin_w(inputs["lr1_w"])
    c["ll1w"] = lin_w(inputs["ll1_w"], ISQ2)
    c["lrgw"] = lin_w(inputs["lrg_w"])
    c["llgw"] = lin_w(inputs["llg_w"], ISQ2)
    c["lr2w"] = lin_w(inputs["lr2_w"])
    c["ll2w"] = lin_w(inputs["ll2_w"], ISQ2)
    c["w1w"], c["w1n"] = gp_w(inputs["w1"], ISQ2)
    c["w2w"], c["w2n"] = gp_w(inputs["w2"], ISQ2)

    # channel-wise GP weights as diagonal matrices [ct, t, 128, 128]
    wg = np.asarray(inputs["wg"], f32) * ISQ2  # [HID, 20]
    dwg = np.zeros((MT_HID, NPATHS, 128, 128), f32)
    idx = np.arange(128)
    for t in range(NPATHS):
        wv = wg[:, t].reshape(MT_HID, 128)
        for ct in range(MT_HID):
            dwg[ct, t, idx, idx] = wv[ct]
    dwg = np.ascontiguousarray(dwg).astype(f16)
    c["dwg"] = dwg
    c["dwgn"] = np.ascontiguousarray(-dwg[:, NEG_TRIPLES])

    cols = []   # list of [128, w] blocks; order must match device PARAM map

    def addp(arr):
        cols.append(np.asarray(arr, f32).reshape(128, -1))

    for a, kt in ((inputs["n1_a"], KT_IN), (inputs["ng_a"], KT_HID),
                  (inputs["n2_a"], KT_HID)):
        sa = sig(a).reshape(kt, 128, 4)
        cb = (1.0 + EPS) - sa
        for u in range(kt):
            addp(sa[u])
            addp(cb[u])
    aa = np.asarray(inputs["act_a"], f32).reshape(MT_HID, 128, 4)
    ab = np.asarray(inputs["act_b"], f32).reshape(MT_HID, 128, 4)
    b1 = (np.asarray(inputs["ll1_b"], f32) * ISQ2).reshape(MT_HID, 128)
    bg = (np.asarray(inputs["llg_b"], f32) * ISQ2).reshape(MT_HID, 128)
    b2 = (np.asarray(inputs["ll2_b"], f32) * ISQ2).reshape(MT_OUT, 128)
    for u in range(MT_HID):
        addp(aa[u])
        addp(ab[u])
        # folded sigmoid biases for the scalar blade: a0*beta + b0
        addp(aa[u][:, 0] * b1[u] + ab[u][:, 0])
        addp(aa[u][:, 0] * bg[u] + ab[u][:, 0])
        addp(b1[u])
        addp(bg[u])
    addp(b2[0])
    c["prm"] = np.ascontiguousarray(np.concatenate(cols, axis=1))

    in_maps = []
    for cid in range(NCORES):
        xc = x[cid * BC:(cid + 1) * BC]  # [BC, 256, 8]
        xt = np.transpose(xc, (1, 2, 0)).reshape(KT_IN, 128, NB, BC)
        m = dict(c)
        m["xT"] = np.ascontiguousarray(xt).astype(f16)  # [kt, 128, 8, BC]
        in_maps.append(m)
    return in_maps


def assemble(results):
    out = np.empty((B, NOUT, NB), np.float32)
    for cid in range(NCORES):
        od = np.asarray(results[cid]["outd"])  # [128, 8, BC]
        out[cid * BC:(cid + 1) * BC] = od.transpose(2, 0, 1)
    return out


# ----------------------------------------------------------------------------
# Device program (identical on all 8 cores)
# ----------------------------------------------------------------------------
def build_program():
    import concourse.mybir as mybir
    import concourse.tile as tile
    from concourse import bacc

    dt = mybir.dt
    AF = mybir.ActivationFunctionType
    OP = mybir.AluOpType

    nc = bacc.Bacc("TRN2", target_bir_lowering=False, debug=False,
                   num_devices=NCORES)

    def din(name, shape, dtype=dt.float16):
        return nc.dram_tensor(name, list(shape), dtype,
                              kind="ExternalInput").ap()

    xT = din("xT", (KT_IN, 128, NB, BC))
    lr1w = din("lr1w", (KT_IN, 4, 128, NIN))
    ll1w = din("ll1w", (KT_IN, 4, 128, HID))
    w1w = din("w1w", (KT_IN, NPATHS, 128, HID))
    w1nd = din("w1n", (KT_IN, NNEG, 128, HID))
    lrgw = din("lrgw", (KT_HID, 4, 128, HID))
    llgw = din("llgw", (KT_HID, 4, 128, HID))
    lr2w = din("lr2w", (KT_HID, 4, 128, HID))
    w2w = din("w2w", (KT_HID, NPATHS, 128, NOUT))
    w2nd = din("w2n", (KT_HID, NNEG, 128, NOUT))
    ll2w = din("ll2w", (KT_HID, 4, 128, NOUT))
    dwg = din("dwg", (MT_HID, NPATHS, 128, 128))
    dwgn = din("dwgn", (MT_HID, NNEG, 128, 128))
    NPRM = 16 + 32 + 32 + 48 + 1
    prm = din("prm", (128, NPRM), dt.float32)
    outd = nc.dram_tensor("outd", [128, NB, BC], dt.float32,
                          kind="ExternalOutput").ap()

    P = lambda j: slice(j * BC, (j + 1) * BC)
    GSL = [slice(j0 * BC, j1 * BC) for (j0, j1) in GRADE_SLICES]

    with tile.TileContext(nc) as tc:
        top = contextlib.ExitStack()
        with top:
            ppool = top.enter_context(tc.tile_pool(name="params", bufs=1))
            npool = top.enter_context(tc.tile_pool(name="nsc", bufs=2))
            qpool = top.enter_context(tc.tile_pool(name="q", bufs=2))
            pspool = top.enter_context(
                tc.tile_pool(name="psum", bufs=4, space="PSUM"))

            # ---------------- params -----------------------------------
            prmt = ppool.tile([128, NPRM], dt.float32, tag="prm", name="prm")
            nc.sync.dma_start(prmt[:], prm)
            PN1, PNG, PN2, PACT, PB2 = 0, 16, 48, 80, 128

            def nprm(base, u):
                sa = {g: prmt[:, base + 8 * u + g:base + 8 * u + g + 1]
                      for g in range(4)}
                cb = {g: prmt[:, base + 8 * u + 4 + g:base + 8 * u + 5 + g]
                      for g in range(4)}
                return sa, cb

            n1p = {u: nprm(PN1, u) for u in range(KT_IN)}
            ngp = {u: nprm(PNG, u) for u in range(KT_HID)}
            n2p = {u: nprm(PN2, u) for u in range(KT_HID)}

            def aprm(u):
                base = PACT + 12 * u
                a = {g: prmt[:, base + g:base + g + 1] for g in range(4)}
                b = {g: prmt[:, base + 4 + g:base + 5 + g] for g in range(4)}
                fh = prmt[:, base + 8:base + 9]   # a0*b1+b0
                fg = prmt[:, base + 9:base + 10]  # a0*bg+b0
                b1 = prmt[:, base + 10:base + 11]
                bg = prmt[:, base + 11:base + 12]
                return a, b, fh, fg, b1, bg

            ap_ = {u: aprm(u) for u in range(MT_HID)}
            b2t = prmt[:, PB2:PB2 + 1]

            # ---------------- weight loading ----------------------------
            def load_lin(pool, name, src, nkt, mtot, split=False):
                t = pool.tile([128, nkt * 4 * mtot], dt.float16,
                              tag=name, name=name)
                if split:
                    for kt in range(nkt):
                        nc.sync.dma_start(
                            t[:, kt * 4 * mtot:(kt + 1) * 4 * mtot].rearrange(
                                "p (a m) -> p a m", a=4),
                            src[kt].rearrange("g p m -> p g m"))
                else:
                    nc.sync.dma_start(
                        t[:].rearrange("p (a m) -> p a m", a=nkt * 4),
                        src.rearrange("kt g p m -> p (kt g) m"))

                def sl(kt, g, mt):
                    base = (kt * 4 + g) * mtot + mt * 128
                    return t[:, base:base + 128]
                return sl

            def load_gp(pool, name, src, srcn, nkt, mtot):
                """GP weights + host-negated copies for NEG_TRIPLES."""
                t = pool.tile([128, nkt * NPATHS * mtot], dt.float16,
                              tag=name, name=name)
                tn = pool.tile([128, nkt * NNEG * mtot], dt.float16,
                               tag=name + "n", name=name + "n")
                for kt in range(nkt):
                    nc.sync.dma_start(
                        t[:, kt * NPATHS * mtot:(kt + 1) * NPATHS * mtot]
                        .rearrange("p (a m) -> p a m", a=NPATHS),
                        src[kt].rearrange("t p m -> p t m"))
                    nc.sync.dma_start(
                        tn[:, kt * NNEG * mtot:(kt + 1) * NNEG * mtot]
                        .rearrange("p (a m) -> p a m", a=NNEG),
                        srcn[kt].rearrange("t p m -> p t m"))

                def sl(kt, tt, mt):
                    base = (kt * NPATHS + tt) * mtot + mt * 128
                    return t[:, base:base + 128]

                def sln(kt, tt, mt):
                    base = (kt * NNEG + NEG_SLOT[tt]) * mtot + mt * 128
                    return tn[:, base:base + 128]
                return sl, sln

            # ---------------- psum bank emitter -------------------------
            class BankEmitter:
                """One [128, 8*BC] f32 psum tile = 2 banks (cols 0:4BC,
                4BC:8BC). Exactly one start per bank (the first matmul);
                stop on the last."""

                def __init__(self, ps, totals):
                    self.ps = ps
                    self.totals = list(totals)  # [nA, nB]
                    self.seen = [0, 0]

                def mm(self, col0, width, lhs, rhs):
                    bank = 0 if col0 < 4 * BC else 1
                    assert (col0 // (4 * BC)) == ((col0 + width - 1) // (4 * BC))
                    i = self.seen[bank]
                    nc.tensor.matmul(self.ps[:, col0:col0 + width], lhs, rhs,
                                     start=(i == 0),
                                     stop=(i == self.totals[bank] - 1),
                                     skip_group_check=True)
                    self.seen[bank] = i + 1

                def done(self):
                    assert self.seen == self.totals, (self.seen, self.totals)

            def alloc_ps(nm):
                return pspool.tile([128, NB * BC], dt.float32, tag="ps",
                                   name=f"ps_{nm}")

            def plane_sel(qpl, o0, L, st):
                if L == 1:
                    return qpl[:, o0:o0 + 1, :]
                last = o0 + st * (L - 1)
                stop = last + 1 if st > 0 else (last - 1 if last >= 1 else None)
                return qpl[:, o0:stop:st, :]

            def build_q(xt, xrt):
                """Mega product tile Q[i*8+k] = x_i * xr_k, [128, 8192]."""
                q = qpool.tile([128, 64 * BC], dt.float16, tag="Q", name="Q")
                for half in range(2):
                    i0 = half * 4
                    a = xt[:, i0 * BC:(i0 + 4) * BC].rearrange(
                        "p (i u b) -> p i u b", i=4, u=1).broadcast_to(
                        [128, 4, 8, BC])
                    bb = xrt[:].rearrange(
                        "p (u k b) -> p u k b", u=1, k=8).broadcast_to(
                        [128, 4, 8, BC])
                    dst = q[:, i0 * 8 * BC:(i0 + 4) * 8 * BC].rearrange(
                        "p (i k b) -> p i k b", i=4, k=8)
                    nc.vector.tensor_mul(dst, a, bb)
                return q

            def emit_lin(em, wsl, xts, nkt, mt):
                for kt in range(nkt):
                    for g in range(4):
                        em.mm(GRADE_SLICES[g][0] * BC, GW[g] * BC,
                              wsl(kt, g, mt), xts[kt][:, GSL[g]])

            def emit_gp_kt(em, wsl, wsln, qpl, kt, mt):
                for (t, (j0, L, o0, st, sgn)) in GP_SETS:
                    lhs = (wsl if sgn > 0 else wsln)(kt, t, mt)
                    em.mm(j0 * BC, L * BC, lhs, plane_sel(qpl, o0, L, st))

            # --- fcgp1: pre-assembled signed plane-combinations (RC tile) --
            # Multi-term / mixed-sign triples get their Cayley sums built on
            # DVE once per kt, so each triple is a single wide matmul.
            DIRECT_T = [0, 1, 2, 3, 5, 13, 19, 16]
            ASM_MM = [(4, 0, 1, 0), (6, 1, 3, 1), (7, 4, 3, 4),
                      (9, 7, 1, 7), (10, 0, 1, 8), (11, 1, 3, 9),
                      (14, 4, 3, 12), (15, 7, 1, 15), (8, 4, 3, 16),
                      (12, 1, 3, 19), (17, 1, 3, 22), (18, 4, 3, 25)]

            def build_rc(q, pool):
                qpl = q[:].rearrange("p (pl b) -> p pl b", pl=64)
                rc = pool.tile([128, 28 * BC], dt.float16, tag="RC",
                               name="RC")
                rv = rc[:].rearrange("p (pl b) -> p pl b", pl=28)
                qs = lambda o: q[:, o * BC:(o + 1) * BC]
                rs = lambda o: rc[:, o * BC:(o + 1) * BC]
                add, sub = nc.vector.tensor_add, nc.vector.tensor_sub
                cp, tsm = nc.vector.tensor_copy, nc.vector.tensor_scalar_mul
                # t4 j0 = Q9+Q18+Q27
                add(rs(0), qs(9), qs(18)); add(rs(0), rs(0), qs(27))
                # t6: j1 = -(Q20+Q29); j2 = Q12-Q30; j3 = Q13+Q22
                add(rs(1), qs(20), qs(29)); tsm(rs(1), rs(1), -1.0)
                sub(rs(2), qs(12), qs(30)); add(rs(3), qs(13), qs(22))
                # t7: j4 = Q10-Q17; j5 = Q11-Q25 (batched); j6 = Q19-Q26
                sub(rv[:, 4:6, :], plane_sel(qpl, 10, 2, 1),
                    plane_sel(qpl, 17, 2, 8))
                sub(rs(6), qs(19), qs(26))
                # t9 j7 = Q14-Q21+Q28
                sub(rs(7), qs(14), qs(21)); add(rs(7), rs(7), qs(28))
                # t10 j0 = -(Q36+Q45+Q54)
                add(rs(8), qs(36), qs(45)); add(rs(8), rs(8), qs(54))
                tsm(rs(8), rs(8), -1.0)
                # t11: j1 = Q34+Q43; j2 = Q51-Q33; j3 = -(Q41+Q50)
                add(rs(9), qs(34), qs(43)); sub(rs(10), qs(51), qs(33))
                add(rs(11), qs(41), qs(50)); tsm(rs(11), rs(11), -1.0)
                # t14: j4 = Q53-Q46; j5 = Q38-Q52 (batched); j6 = Q44-Q37
                sub(rv[:, 12:14, :], plane_sel(qpl, 53, 2, -15),
                    plane_sel(qpl, 46, 2, 6))
                sub(rs(14), qs(44), qs(37))
                # t15 j7 = Q35-Q42+Q49
                sub(rs(15), qs(35), qs(42)); add(rs(15), rs(15), qs(49))
                # t8: j4 = +Q31, j5 = -Q23, j6 = +Q15
                cp(rv[:, 16:19:2, :], plane_sel(qpl, 31, 2, -16))
                tsm(rs(17), qs(23), -1.0)
                # t12: j1 = -Q55, j2 = +Q47, j3 = -Q39
                cp(rs(20), qs(47))
                tsm(rv[:, 19:22:2, :], plane_sel(qpl, 55, 2, -16), -1.0)
                # t17: j1 = -Q62, j2 = +Q61, j3 = -Q60
                cp(rs(23), qs(61))
                tsm(rv[:, 22:25:2, :], plane_sel(qpl, 62, 2, -2), -1.0)
                # t18: j4 = +Q59, j5 = -Q58, j6 = +Q57
                cp(rv[:, 25:28:2, :], plane_sel(qpl, 59, 2, -2))
                tsm(rs(26), qs(58), -1.0)
                return rc

            def emit_gp_rc(em, wsl, wsln, qpl, rc, kt, mt):
                for t in DIRECT_T:
                    (j0, L, o0, st, sgn) = TERM_SETS[t][0]
                    lhs = (wsl if sgn > 0 else wsln)(kt, t, mt)
                    em.mm(j0 * BC, L * BC, lhs, plane_sel(qpl, o0, L, st))
                rcv = rc[:].rearrange("p (pl b) -> p pl b", pl=28)
                for (t, j0, L, base) in ASM_MM:
                    em.mm(j0 * BC, L * BC, wsl(kt, t, mt),
                          rcv[:, base:base + L, :])

            LIN_BANK = [sum(GW[:2]), sum(GW[2:])]  # grades per bank: 4, 4
            GPA = sum(1 for (t, s) in GP_SETS if s[0] < 4)   # sets in bank A
            GPB = len(GP_SETS) - GPA

            # ---------------- fused activation chains -------------------
            def norm_pair(pss, outs, prms):
                """normalize 2 tiles: out = ps / (sa*sqrt(q)+cb)."""
                n = len(pss)
                assert n == 2
                sqw = npool.tile([128, 12 * BC], dt.float16, tag="sqw",
                                 name="sqw")
                q12 = npool.tile([128, 4 * BC], dt.float16, tag="q12",
                                 name="q12")
                nrm = npool.tile([128, 8 * BC], dt.float32, tag="nrm",
                                 name="nrm", bufs=1)
                rw = npool.tile([128, 8 * BC], dt.float32, tag="rw",
                                name="rw", bufs=1)
                nv = nrm[:].rearrange("p (pl b) -> p pl b", pl=8)
                for i, ps in enumerate(pss):
                    # squares of blades 1..6 (grades 1,2)
                    nc.scalar.activation(sqw[:, i * 6 * BC:(i + 1) * 6 * BC],
                                         ps[:, BC:7 * BC], AF.Square)
                    # norms for 1-blade grades 0,3: |x|
                    psv = ps[:].rearrange("p (pl b) -> p pl b", pl=8)
                    nc.scalar.activation(nv[:, 4 * i:4 * i + 4:3, :],
                                         psv[:, 0:8:7, :], AF.Abs)
                sqp = sqw[:].rearrange("p (pl b) -> p pl b", pl=12)
                qv = q12[:].rearrange("p (pl b) -> p pl b", pl=4)
                nc.vector.tensor_add(qv, sqp[:, 0:10:3, :], sqp[:, 1:11:3, :])
                nc.vector.tensor_add(qv, qv, sqp[:, 2:12:3, :])
                # norms for grades 1,2 = sqrt(q12)
                nvt = nrm[:].rearrange("p (t g b) -> p t g b", t=2, g=4)
                qvt = q12[:].rearrange("p (t g b) -> p t g b", t=2, g=2)
                nc.scalar.activation(nvt[:, :, 1:3, :], qvt, AF.Sqrt)
                # dw = sa*nrm + cb (per tile+grade; in-place on nrm)
                for i in range(n):
                    sa, cb = prms[i]
                    for g in range(4):
                        s = slice((4 * i + g) * BC, (4 * i + g + 1) * BC)
                        nc.vector.tensor_scalar(nrm[:, s], nrm[:, s],
                                                sa[g], cb[g],
                                                OP.mult, OP.add)
                nc.vector.reciprocal_approx_fast(rw[:], nrm[:])
                rv = rw[:].rearrange("p (t g b) -> p t g b", t=2, g=4)
                for i, ps in enumerate(pss):
                    ov = outs[i][:].rearrange("p (pl b) -> p pl b", pl=8)
                    psv = ps[:].rearrange("p (pl b) -> p pl b", pl=8)
                    nc.vector.tensor_mul(ov[:, 0:8:7, :], psv[:, 0:8:7, :],
                                         rv[:, i, 0:4:3, :])
                    bb = rw[:, (4 * i + 1) * BC:(4 * i + 3) * BC].rearrange(
                        "p (g u b) -> p g u b", g=2, u=1).broadcast_to(
                        [128, 2, 3, BC])
                    nc.vector.tensor_mul(
                        outs[i][:, BC:7 * BC].rearrange(
                            "p (g u b) -> p g u b", g=2, u=3),
                        pss[i][:, BC:7 * BC].rearrange(
                            "p (g u b) -> p g u b", g=2, u=3), bb)

            def silu_pair(pss, outs, prms, phase):
                """mv_silu 2 tiles; blade-0 bias folded via scale/bias."""
                n = len(pss)
                assert n == 2
                sqw = npool.tile([128, 14 * BC], dt.float16, tag="sqs",
                                 name="sqs")
                q12 = npool.tile([128, 4 * BC], dt.float16, tag="q12s",
                                 name="q12s")
                gw = npool.tile([128, 8 * BC], dt.float16, tag="gw",
                                name="gw")
                for i, ps in enumerate(pss):
                    # squares of blades 1..7
                    nc.scalar.activation(sqw[:, i * 7 * BC:(i + 1) * 7 * BC],
                                         ps[:, BC:8 * BC], AF.Square)
                sqt = sqw[:].rearrange("p (t pl b) -> p t pl b", t=2, pl=7)
                qq = q12[:].rearrange("p (t g b) -> p t g b", t=2, g=2)
                nc.vector.tensor_add(qq, sqt[:, :, 0:4:3, :],
                                     sqt[:, :, 1:5:3, :])
                nc.vector.tensor_add(qq, qq, sqt[:, :, 2:6:3, :])
                for i, ps in enumerate(pss):
                    a, b, fh, fg, b1, bg = prms[i]
                    f0 = fh if phase == 0 else fg
                    bb0 = b1 if phase == 0 else bg
                    go = 4 * i * BC
                    # gate0 = sigmoid(a0*(ps0 + beta) + b0)
                    nc.scalar.activation(gw[:, go:go + BC], ps[:, 0:BC],
                                         AF.Sigmoid, bias=f0, scale=a[0])
                    for g in (1, 2):
                        nc.scalar.activation(
                            gw[:, go + g * BC:go + (g + 1) * BC],
                            q12[:, (2 * i + g - 1) * BC:(2 * i + g) * BC],
                            AF.Sigmoid, bias=b[g], scale=a[g])
                    nc.scalar.activation(
                        gw[:, go + 3 * BC:go + 4 * BC],
                        sqw[:, (7 * i + 6) * BC:(7 * i + 7) * BC],
                        AF.Sigmoid, bias=b[3], scale=a[3])
                    # h0 = (ps0 + beta) * gate0
                    nc.vector.scalar_tensor_tensor(
                        outs[i][:, 0:BC], ps[:, 0:BC], bb0,
                        gw[:, go:go + BC], OP.add, OP.mult)
                    gv = gw[:, go + BC:go + 3 * BC].rearrange(
                        "p (g u b) -> p g u b", g=2, u=1).broadcast_to(
                        [128, 2, 3, BC])
                    nc.vector.tensor_mul(
                        outs[i][:, BC:7 * BC].rearrange(
                            "p (g u b) -> p g u b", g=2, u=3),
                        ps[:, BC:7 * BC].rearrange(
                            "p (g u b) -> p g u b", g=2, u=3), gv)
                    nc.vector.tensor_mul(outs[i][:, 7 * BC:8 * BC],
                                         ps[:, 7 * BC:8 * BC],
                                         gw[:, go + 3 * BC:go + 4 * BC])

            # ============ weight pools & prefetch ==========================
            hpool = top.enter_context(tc.tile_pool(name="hacts", bufs=1))
            Ht, H2t = {}, {}
            with tc.tile_pool(name="w_s3", bufs=1) as w3pool:
                with tc.tile_pool(name="w_s1", bufs=1) as w1pool, \
                     tc.tile_pool(name="xacts", bufs=1) as xpool:
                    # phase 1-2 inputs + weights first (DMA need-order)
                    Xt = {}
                    for kt in range(KT_IN):
                        t = xpool.tile([128, NB * BC], dt.float16,
                                       tag=f"X_{kt}", name=f"X_{kt}")
                        nc.sync.dma_start(
                            t[:].rearrange("p (i b) -> p i b", i=NB), xT[kt])
                        Xt[kt] = t
                    lr1t = load_lin(w1pool, "lr1", lr1w, KT_IN, NIN)
                    ll1t = load_lin(w1pool, "ll1", ll1w, KT_IN, HID)
                    w1t, w1nt = load_gp(w1pool, "w1", w1w, w1nd, KT_IN, HID)
                    # phase 3-4 linear weights (prefetch; disjoint region)
                    lrgt = load_lin(w3pool, "lrg", lrgw, KT_HID, HID)
                    llgt = load_lin(w3pool, "llg", llgw, KT_HID, HID)

                    # ======== phase 1: xr = normalization(lr1(x)) ==========
                    with tc.tile_pool(name="xracts", bufs=1) as xrpool:
                        XRt = {}
                        pss = []
                        for mt in range(MT_IN):
                            ps = alloc_ps(f"lr1_{mt}")
                            em = BankEmitter(ps, [KT_IN * 2, KT_IN * 2])
                            emit_lin(em, lr1t, Xt, KT_IN, mt)
                            em.done()
                            pss.append(ps)
                            XRt[mt] = xrpool.tile([128, NB * BC], dt.float16,
                                                  tag=f"XR_{mt}",
                                                  name=f"XR_{mt}")
                        norm_pair(pss, [XRt[0], XRt[1]],
                                  [(n1p[0][0], n1p[0][1]),
                                   (n1p[1][0], n1p[1][1])])

                        # ======== phase 2: h = silu(ll1(x) + fcgp) =========
                        Qs = {kt: build_q(Xt[kt], XRt[kt])
                              for kt in range(KT_IN)}
                        RCs = {kt: build_rc(Qs[kt], xrpool)
                               for kt in range(KT_IN)}
                        Qpl = {kt: Qs[kt][:].rearrange(
                            "p (pl b) -> p pl b", pl=64) for kt in range(KT_IN)}
                        ems = {}
                        psH = {}
                        for mt in range(MT_HID):
                            ps = alloc_ps(f"h_{mt}")
                            em = BankEmitter(ps, [KT_IN * 12, KT_IN * 12])
                            emit_lin(em, ll1t, Xt, KT_IN, mt)
                            ems[mt], psH[mt] = em, ps
                        for mt in range(MT_HID):
                            for kt in range(KT_IN):
                                emit_gp_rc(ems[mt], w1t, w1nt, Qpl[kt],
                                           RCs[kt], kt, mt)
                            ems[mt].done()
                        for mt in range(MT_HID):
                            Ht[mt] = hpool.tile([128, NB * BC], dt.float16,
                                                tag=f"H_{mt}", name=f"H_{mt}")
                        for p0 in range(0, MT_HID, 2):
                            silu_pair([psH[p0], psH[p0 + 1]],
                                      [Ht[p0], Ht[p0 + 1]],
                                      [ap_[p0], ap_[p0 + 1]], phase=0)

                # ======== phase 3: hr = normalization(lrg(h)) =============
                with tc.tile_pool(name="w_s3b", bufs=1) as w3bpool, \
                     tc.tile_pool(name="hracts", bufs=1) as hrpool:
                    # cw-GP diag weights load once phase-1/2 space frees
                    dt_ = w3bpool.tile([128, MT_HID * NPATHS * 128],
                                       dt.float16, tag="dwg", name="dwg")
                    nc.sync.dma_start(
                        dt_[:].rearrange("p (a m) -> p a m",
                                         a=MT_HID * NPATHS),
                        dwg.rearrange("c t p m -> p (c t) m"))
                    dn_ = w3bpool.tile([128, MT_HID * NNEG * 128], dt.float16,
                                       tag="dwgn", name="dwgn")
                    nc.sync.dma_start(
                        dn_[:].rearrange("p (a m) -> p a m",
                                         a=MT_HID * NNEG),
                        dwgn.rearrange("c t p m -> p (c t) m"))

                    def dwgsl(ct, t):
                        base = (ct * NPATHS + t) * 128
                        return dt_[:, base:base + 128]

                    def dwgsln(ct, t):
                        base = (ct * NNEG + NEG_SLOT[t]) * 128
                        return dn_[:, base:base + 128]

                    HRt = {}
                    for p0 in range(0, MT_HID, 2):
                        pss = []
                        for mt in (p0, p0 + 1):
                            ps = alloc_ps(f"lrg_{mt}")
                            em = BankEmitter(ps, [KT_HID * 2, KT_HID * 2])
                            emit_lin(em, lrgt, Ht, KT_HID, mt)
                            em.done()
                            pss.append(ps)
                            HRt[mt] = hrpool.tile([128, NB * BC], dt.float16,
                                                  tag=f"HR_{mt}",
                                                  name=f"HR_{mt}")
                        norm_pair(pss, [HRt[p0], HRt[p0 + 1]],
                                  [(ngp[p0][0], ngp[p0][1]),
                                   (ngp[p0 + 1][0], ngp[p0 + 1][1])])

                    # ======== phase 4: h2 = silu(llg(h) + cw_gp) ===========
                    psH2 = {}
                    for mt in range(MT_HID):
                        ps = alloc_ps(f"h2_{mt}")
                        em = BankEmitter(
                            ps, [KT_HID * 2 + GPA, KT_HID * 2 + GPB])
                        emit_lin(em, llgt, Ht, KT_HID, mt)
                        q = build_q(Ht[mt], HRt[mt])
                        qpl = q[:].rearrange("p (pl b) -> p pl b", pl=64)
                        for (t, (j0, L, o0, st, sgn)) in GP_SETS:
                            lhs = (dwgsl if sgn > 0 else dwgsln)(mt, t)
                            em.mm(j0 * BC, L * BC, lhs,
                                  plane_sel(qpl, o0, L, st))
                        em.done()
                        psH2[mt] = ps
                        H2t[mt] = hpool.tile([128, NB * BC], dt.float16,
                                             tag=f"H2_{mt}", name=f"H2_{mt}")
                    for p0 in range(0, MT_HID, 2):
                        silu_pair([psH2[p0], psH2[p0 + 1]],
                                  [H2t[p0], H2t[p0 + 1]],
                                  [ap_[p0], ap_[p0 + 1]], phase=1)

            # ============ phases 5-6 ======================================
            w2pool = top.enter_context(tc.tile_pool(name="w_s2", bufs=1))
            lr2t = load_lin(w2pool, "lr2", lr2w, KT_HID, HID)
            ll2t = load_lin(w2pool, "ll2", ll2w, KT_HID, NOUT)
            w2t, w2nt = load_gp(w2pool, "w2", w2w, w2nd, KT_HID, NOUT)
            # all four lr2 psum groups up front, then per-pair:
            # norm -> Q builds -> GP matmuls, so fcgp2 starts as soon as
            # the first pair of HR2 tiles is ready.
            HR2t, ps2, pair_ps = {}, {}, {}
            for u in range(MT_HID):
                ps = alloc_ps(f"lr2_{u}")
                em = BankEmitter(ps, [KT_HID * 2, KT_HID * 2])
                emit_lin(em, lr2t, H2t, KT_HID, u)
                em.done()
                ps2[u] = ps
                HR2t[u] = w2pool.tile([128, NB * BC], dt.float16,
                                      tag=f"HR2_{u}", name=f"HR2_{u}")
            norm_pair([ps2[0], ps2[1]], [HR2t[0], HR2t[1]],
                      [(n2p[0][0], n2p[0][1]), (n2p[1][0], n2p[1][1])])

            # out = (ll2(h2) + fcgp(h2, hr2, w2)) / sqrt2
            ps = alloc_ps("out")
            em = BankEmitter(ps, [KT_HID * (2 + GPA), KT_HID * (2 + GPB)])
            emit_lin(em, ll2t, H2t, KT_HID, 0)
            for kt in (0, 1):
                q = build_q(H2t[kt], HR2t[kt])
                qpl = q[:].rearrange("p (pl b) -> p pl b", pl=64)
                emit_gp_kt(em, w2t, w2nt, qpl, kt, 0)
            norm_pair([ps2[2], ps2[3]], [HR2t[2], HR2t[3]],
                      [(n2p[2][0], n2p[2][1]), (n2p[3][0], n2p[3][1])])
            for kt in (2, 3):
                q = build_q(H2t[kt], HR2t[kt])
                qpl = q[:].rearrange("p (pl b) -> p pl b", pl=64)
                emit_gp_kt(em, w2t, w2nt, qpl, kt, 0)
            em.done()
            outs = npool.tile([128, NB * BC], dt.float32, tag="outs",
                              name="outs", bufs=1)
            nc.scalar.activation(outs[:, 0:BC], ps[:, 0:BC],
                                 AF.Identity, bias=b2t)
            nc.scalar.copy(outs[:, BC:], ps[:, BC:])
            nc.sync.dma_start(outd[0:128],
                              outs[:].rearrange("p (i b) -> p i b", i=NB))

    nc.compile()
    return nc


_PROGRAM = None


def _get_program():
    global _PROGRAM
    if _PROGRAM is None:
        _PROGRAM = build_program()
    return _PROGRAM


def kernel(**inputs):
    from concourse.bass_utils import run_bass_kernel_spmd

    nc = _get_program()
    in_maps = prep_in_maps(inputs)
    res = run_bass_kernel_spmd(nc, in_maps, core_ids=list(range(NCORES)))
    return assemble(res.results)


if __name__ == "__main__":
    nmm = sum(len(TERM_SETS[t]) for t in range(NPATHS))
    print("NEG_TRIPLES:", NEG_TRIPLES)
    print("term-set MMs per (kt,mt):", nmm)
